# revision 4
# baseline (speedup 1.0000x reference)
"""Trainium2 Bass kernel for nn_BatchGraphEncoder (gnn_message_passing).

Math note: the reference's segment softmax uses B unique segment ids
(groups of size 1), so alpha == exp(x-x)/1 == 1.0 bit-exactly for any
finite scores.  The output is therefore independent of the attention
inputs (w_i, w_j, w_k) and reduces to pure batch sums:

    out[:,   0:128] = sum_b h[b,:]      (broadcast over the N=512 rows)
    out[:, 128:256] = sum_b r[b,:]      (broadcast)
    out[:, 256:384] = sum_b t[b,:,:]    ([512, 128])

This is a memory-bound reduction over B=2048 dominated by reading t
(512 MB).  Strategy: shard B across the 8 cores (data parallel),
reduce over the local batch on-device, and sum the 8 partials on the
host.  h/r sums (1 MB each) are computed on the host outright.

Layout: the [B_loc, 65536] shard is viewed as [NCH, 128, 4096] so each
8-batch-row chunk IS a [128, 4096] DRAM slab: partition p = 16*bi + q
holds row (8k+bi), flat columns [4096q, 4096q+4096).  Each chunk DMA is
a plain 2 MB block copy with 16 KB contiguous runs - 128 descriptors
per chunk.  Descriptor size matters a lot here: 2 KB runs cap the
per-core stream at ~379 GB/s while 16 KB runs reach ~420 GB/s (near
the 435 GB/s SBUF-AXI fabric ceiling).

On-device reduction rides the otherwise-idle TensorEngine: each chunk
is folded into a PSUM accumulator psum[16, 4096] (all 8 banks of
partitions 0-15) by 8 bank-matmuls against a [128, 16] 0/1 stationary
W[p, q] = (p % 16 == q), viewed as float32r (single-pass TF32-ish
matmul: 1 cycle/row at free>=256, ~1.8 us/chunk vs the ~4.8 us chunk
DMA; full fp32 would be 4 cycles/row and throttle the stream).  The
fold sums the 8 batch rows of a chunk AND the across-chunk axis in one
accumulator, so the kernel output is only [16, 4096] (256 KB, 8x
smaller than the DVE-merge design's [128, 4096]) - the previous
design's 2 MB output write drained ~7.5 us past the end of the read
stream; this one is ~2 us.  psum[q, j] = sum_b t_loc[b, 4096q+j], so
the host just sums the 8 per-core [16, 4096] partials (f64) and
reshapes flat -> [N, D].  No DVE work at all mid-stream.

The whole t stream rides the SP HWDGE ring so chunks complete strictly
in order; the ACT ring carries only the tiny stationary load and the
final 256 KB output.  The last chunk is streamed as four column-quarter
DMAs so its bank-matmuls, the PSUM->SBUF bank copies (DVE/ACT
alternating), and the two output-half DMAs pipeline against the final
quarters' arrival.

Load balancing: per-core bandwidth episodes rove between cores on a
minutes timescale, but profiling shows a stable asymmetry: odd phys
NCs hold ~417 GB/s in virtually every run while even phys NCs
(logical 0/2/4/6 here) drop to 330-400 GB/s in frequent episodes,
phys NC 0 (logical 6) nearly always.  Shards are sized for those
reliability classes.  Rows a core does not own are skipped via
conditional DMAs (sync-engine pid predicate); their stale-buffer
matmuls are neutralized by per-tier stationaries that the HOST zeroes
on skipping cores (the stationaries are per-core kernel inputs - no
on-device mask computation, no tc.If chains; stale SBUF is finite so
x*0 == 0 exactly).  Emission order keeps the first 8 and last chunk
slots unconditional so a pool buffer's first use is never a skipped
DMA (stale-finite guarantee) and the quarter-split tail chunk runs on
every core.
"""

import numpy as np

B, N, D = 2048, 512, 128
NCORES = 8
FLAT = N * D                 # 65536 flattened (n, d) columns
CW = 4096                    # chunk free width = 16 KB descriptor runs
RPC = 8                      # batch rows per chunk (8 * 16 partitions)

# Per-core row counts, proportional to observed reliability: odd phys
# NCs (logical 1, 3, 5, 7) sustain ~417 GB/s in virtually every run;
# even phys NCs (logical 0, 2, 4, 6) drop to 330-400 GB/s in frequent
# episodes, with phys NC 0 (logical 6) the worst and most persistent.
SIZES = [240, 280, 240, 272, 240, 280, 216, 280]
B_MAX = max(SIZES)           # 280
NCH = B_MAX // RPC           # 35 chunks of [128, 4096] = 2 MB
assert sum(SIZES) == B and all(s % RPC == 0 for s in SIZES)

# chunk c covers rows [8c, 8c+8); cores with SIZES <= 8c skip it.
_SKIP = {c: tuple(p for p in range(NCORES) if SIZES[p] <= RPC * c)
         for c in range(NCH)}
COND = {c: s for c, s in _SKIP.items() if s}          # chunk -> skip pids
TIERS = sorted(set(COND.values()))                    # distinct skip sets
NT = len(TIERS)

# Emission order: first 8 and the last slots unconditional (a
# conditional chunk must never be a pool buffer's first use - the
# stationary-zeroed matmul of a skipped DMA would read uninitialized
# SBUF, and 0 * NaN == NaN), conditionals mid-stream.
_UNC = [c for c in range(NCH) if c not in COND]
_CND = sorted(COND)
ORDER = list(_UNC[:8])
_rest = _UNC[8:]
for i, c in enumerate(_CND):
    ORDER.append(c)
    ORDER.extend(_rest[2 * i: 2 * i + 2])
ORDER.extend(_rest[2 * len(_CND):])
assert sorted(ORDER) == list(range(NCH))
assert all(c not in COND for c in ORDER[:8]) and ORDER[-1] not in COND

_BUILT = None
# test.py can inject {"trace": True, ...} here; harness path leaves it empty.
RUN_KWARGS = {}
LAST_RESULTS = None


def _build():
    from concourse import bacc, tile, mybir

    f32 = mybir.dt.float32
    f32r = mybir.dt.float32r
    nc = bacc.Bacc(
        "TRN2",
        target_bir_lowering=False,
        debug=False,
        enable_asserts=False,
        num_devices=NCORES,
    )
    t_in = nc.dram_tensor("t_shard", [NCH, 128, CW], f32r,
                          kind="ExternalInput").ap()
    # Column block i of 16 = stationary for tier i-1 (block 0 =
    # unconditional): W[p, q] = (p % 16 == q), zeroed by the HOST on
    # cores that skip the tier.  Per-core input values do the masking.
    w_in = nc.dram_tensor("w_tiers", [128, 16 * (1 + NT)], f32r,
                          kind="ExternalInput").ap()
    out_t = nc.dram_tensor("out_t_part", [16, CW], f32,
                           kind="ExternalOutput").ap()

    with tile.TileContext(nc) as tc:
        with (
            tc.tile_pool(name="wconst", bufs=1) as wpool,
            tc.tile_pool(name="loads", bufs=8) as loads,
            tc.tile_pool(name="res", bufs=1) as res,
            tc.tile_pool(name="acc", bufs=1, space="PSUM") as ppool,
        ):
            wt = wpool.tile([128, 16 * (1 + NT)], f32r)
            psum_t = ppool.tile([16, CW], f32)
            stage = res.tile([16, CW], f32)
            skip_cond = {}

            def wsel(c):
                i = (1 + TIERS.index(COND[c])) if c in COND else 0
                return wt[:, 16 * i: 16 * i + 16]

            # The stationary load MUST precede the first chunk's
            # matmuls in program order (a later write would be a WAR
            # dep: the matmuls would run first, on uninitialized SBUF).
            # It rides the ACT ring so the SP ring's first chunk DMA
            # issue is not delayed.
            nc.scalar.dma_start(wt[:], w_in[:])

            def emit_setup():
                # Emitted after the first few t loads so the pid
                # register load (a ~950 ns sync-engine TENSOR_LOAD)
                # never delays the t stream's start; it is only needed
                # before the first conditional chunk at position 8.
                pid_sync = nc.sync.partition_id()
                for t in TIERS:
                    cs = None
                    for p in t:
                        es = pid_sync != p
                        cs = es if cs is None else cs * es
                    skip_cond[t] = cs

            # --- t batch sum: 8 bank-matmuls per 2 MB chunk ---
            for k, c in enumerate(ORDER):
                if k == 3:
                    emit_setup()
                if k == NCH - 1:
                    # Last chunk: stream as four column-quarter DMAs;
                    # each quarter's two bank-matmuls chase its
                    # arrival, the PSUM->SBUF bank copies (DVE/ACT
                    # alternating) chase the matmuls, and the two
                    # output-half DMAs chase the copies, so only the
                    # final quarter's matmul + copy + 128 KB write +
                    # receipt stay exposed past the stream's last byte.
                    q = CW // 4
                    for j in range(4):
                        sl = slice(j * q, (j + 1) * q)
                        tq = res.tile([128, q], f32r, name=f"tq{j}")
                        nc.sync.dma_start(tq[:], t_in[c][:, sl])
                        for b2 in range(2):
                            bank = slice((2 * j + b2) * 512,
                                         (2 * j + b2 + 1) * 512)
                            nc.tensor.matmul(
                                psum_t[:, bank],
                                wsel(c),
                                tq[:, b2 * 512: (b2 + 1) * 512],
                                start=False,
                                stop=True,
                            )
                            eng = nc.vector if b2 == 0 else nc.scalar
                            if b2 == 0:
                                nc.vector.tensor_copy(
                                    stage[:, bank], psum_t[:, bank])
                            else:
                                nc.scalar.copy(
                                    out=stage[:, bank], in_=psum_t[:, bank])
                        if j == 1:
                            nc.scalar.dma_start(
                                out_t[:, 0: CW // 2], stage[:, 0: CW // 2])
                        if j == 3:
                            nc.scalar.dma_start(
                                out_t[:, CW // 2:], stage[:, CW // 2:])
                    continue
                tl = loads.tile([128, CW], f32r)
                if c in COND:
                    # Skipped on cores not owning these rows: the slot
                    # then holds stale (finite) data from an earlier
                    # chunk; the host-zeroed tier stationary
                    # neutralizes it (x * 0 == 0).
                    nc.sync.dma_start(tl[:], t_in[c], cond=skip_cond[COND[c]])
                else:
                    nc.sync.dma_start(tl[:], t_in[c])
                for b in range(8):
                    bank = slice(b * 512, (b + 1) * 512)
                    nc.tensor.matmul(
                        psum_t[:, bank],
                        wsel(c),
                        tl[:, bank],
                        start=(k == 0),
                        stop=False,
                    )

    nc.compile()
    return nc


def _get_built():
    global _BUILT
    if _BUILT is None:
        _BUILT = _build()
    return _BUILT


def kernel(h, r, t, w_i, w_j, w_k):
    global LAST_RESULTS
    from concourse import bass_utils

    nc = _get_built()
    t2 = np.ascontiguousarray(t, dtype=np.float32).reshape(B, FLAT)

    # Fold stationary: W[p, q] = (p % 16 == q).
    w_eye = np.zeros((128, 16), dtype=np.float32)
    w_eye[np.arange(128), np.arange(128) % 16] = 1.0

    starts = np.concatenate([[0], np.cumsum(SIZES)])
    in_maps = []
    for c in range(NCORES):
        s, e = int(starts[c]), int(starts[c + 1])
        tc_ = t2[s:e]
        if e - s < B_MAX:
            tc_ = np.concatenate(
                [tc_, np.zeros((B_MAX - (e - s), FLAT), dtype=np.float32)])
        w_tiers = np.concatenate(
            [w_eye] + [w_eye * (0.0 if c in t else 1.0) for t in TIERS],
            axis=1,
        )
        in_maps.append(
            {
                "t_shard": np.ascontiguousarray(tc_).reshape(NCH, 128, CW),
                "w_tiers": np.ascontiguousarray(w_tiers),
            }
        )
    results = bass_utils.run_bass_kernel_spmd(
        nc, in_maps, core_ids=list(range(NCORES)), **RUN_KWARGS
    )
    LAST_RESULTS = results

    # psum[q, j] = sum_b t_loc[b, 4096q + j]; finish the sum over cores
    # on the host (f64), then unflatten to [N, D].  h/r sums are pure
    # host work (1 MB each vs t's 512 MB).
    sum_t = np.zeros((16, CW), dtype=np.float64)
    for c in range(NCORES):
        sum_t += results.results[c]["out_t_part"]
    t_full = sum_t.reshape(FLAT).reshape(N, D)
    sum_h = h.astype(np.float64).sum(axis=0)
    sum_r = r.astype(np.float64).sum(axis=0)

    out = np.empty((N, 3 * D), dtype=np.float32)
    out[:, 0:D] = sum_h.astype(np.float32)[None, :]
    out[:, D: 2 * D] = sum_r.astype(np.float32)[None, :]
    out[:, 2 * D:] = t_full.astype(np.float32)
    return out


# revision 20
# speedup vs baseline: 1.0822x; 1.0822x over previous
"""Trainium2 Bass kernel for nn_BatchGraphEncoder (gnn_message_passing).

Math note: the reference's segment softmax uses B unique segment ids
(groups of size 1), so alpha == exp(x-x)/1 == 1.0 bit-exactly for any
finite scores.  The output is therefore independent of the attention
inputs (w_i, w_j, w_k) and reduces to pure batch sums:

    out[:,   0:128] = sum_b h[b,:]      (broadcast over the N=512 rows)
    out[:, 128:256] = sum_b r[b,:]      (broadcast)
    out[:, 256:384] = sum_b t[b,:,:]    ([512, 128])

This is a memory-bound reduction over B=2048 dominated by reading t
(512 MB).  Strategy: shard B across the 8 cores (data parallel),
reduce over the local batch on-device, and sum the 8 partials on the
host.  h/r sums (1 MB each) are computed on the host outright.

Layout: the [B_loc, 65536] shard is viewed as [NCH, 128, 8192] so each
16-batch-row chunk IS a [128, 8192] DRAM slab (4 MB): partition
p = 8*ri + q holds row (16k+ri), flat columns [8192q, 8192q+8192).
Each chunk DMA is a plain block copy with 32 KB contiguous runs - 128
descriptors per chunk.  Descriptor size matters a lot here: 2 KB runs
cap the per-core stream at ~379 GB/s, 16 KB runs ~417 GB/s; 32 KB gets
closer to the 435 GB/s SBUF-AXI fabric ceiling.  Halving the DMA count
also halves the Tile semaphore footprint, which sets the length of the
fixed all-engine sem-drain at kernel exit (~115 ns per allocated sem
on the slowest engine - it was ~8 us of the previous design's tail).

On-device reduction rides the otherwise-idle TensorEngine: each chunk
is folded into a PSUM accumulator psum[32, 2048] by 16 bank-matmuls
against [128, 32] 0/1 stationaries, viewed as float32r (single-pass
TF32-ish matmul, ~0.45 us per 512-wide matmul incl. weight reload,
~7.2 us/chunk vs the ~9.4 us chunk DMA; full fp32 would be 4
cycles/row and throttle the stream).  The stationary for column block
q2 (j in [2048*q2, 2048*q2+2048)) is W[p, m] = (m == 4*(p%8) + q2), so
psum[m, u] = sum_b t_loc[b, 2048*m + u] - i.e. psum IS the flat batch
sum, [32, 2048] = 256 KB (vs 2 MB for a DVE-merge design, whose output
write drained ~7.5 us past the end of the read stream).  fp32r
truncates the t mantissas (~1e-5 l2 error vs the 2e-2 gate); PSUM
accumulates fp32.

The whole t stream rides the SP HWDGE ring so chunks complete strictly
in order; the ACT ring carries only the tiny stationary load and the
final output.  PSUM cannot be DMA'd directly, so bank b of psum
([32, 512]) is copied (DVE banks 0-1, ACT banks 2-3, in parallel) to a
[128, 512] staging tile at partition group 32b - engine writes must
start at a 32-aligned partition (the BIR verifier rejects partition
offset 16), and the staging DMA then spans all 128 partitions / 16 AXI
ports (a [32, N] tile sits on 8).  The last chunk is streamed as
three 1 MB quarter DMAs (q2 blocks 0-2) plus two 512 KB eighths for
q2=3, so each psum bank is final right after its q2=3 matmul; the
copies chase the eighths and the four [32, 512] output DMAs ride the
then-idle SP ring, leaving only ~1 us of data movement exposed past
the stream's last byte (measured post-stream time: ~8.6 us, almost
all of it the framework's fixed ~57-semaphore exit drain).

Load balancing: per-run aggregate bandwidth is ~3.2 TB/s and cores
split it unevenly; in calm windows odd phys NCs (logical 1, 3, 5, 7)
sustain ~417-420 GB/s, logical 0/2/4 ~400-420, while logical 6 (phys
NC 0) drops to 330-370 GB/s nearly always; in busy windows the slow
episode roves across cores (min-over-repeated-runs samples past it).
Shards are sized for those classes (16-row granularity).  Rows a core does not own are skipped
via conditional DMAs (sync-engine pid predicate); their stale-buffer
matmuls are neutralized by per-tier stationaries that the HOST zeroes
on skipping cores (the stationaries are per-core kernel inputs - no
on-device mask computation; stale SBUF is finite so x*0 == 0).
Emission order keeps the first 8 and the last chunk slots
unconditional so a pool buffer's first use is never a skipped DMA
(uninitialized SBUF could be NaN, and 0 * NaN == NaN) and the
quarter-split tail chunk carries real data on every core.
"""

import numpy as np

B, N, D = 2048, 512, 128
NCORES = 8
FLAT = N * D                 # 65536 flattened (n, d) columns
CW = 8192                    # chunk free width = 32 KB descriptor runs
RPC = 16                     # batch rows per chunk (16 * 8 partitions)

# Per-core row counts by reliability class: in clean windows all
# non-6 cores sustain ~418-420 GB/s; logical 6 (phys NC 0) holds
# ~330-370 nearly always.  Every core ends with an 8-row MINI chunk
# (2 MB, 16 KB-run layout, own fold stationaries, per-core source
# rows) that breaks the 16-row granularity of the 4 MB chunks: seven
# cores carry 264 rows (16 chunks + mini) = 164.8 us at 420 GB/s
# (the best pure-16-row split bottoms out at 272 rows = 169.9 us),
# and logical 6 carries 200 (12 chunks + mini) = 169.7 us at its
# typical 330 GB/s, keeping it just under the critical path.
SIZES = [264, 264, 264, 264, 264, 264, 200, 264]
B_FULL = 256                 # rows covered by full 16-row chunks
NCH = B_FULL // RPC          # 16 chunks of [128, 8192] = 4 MB
assert sum(SIZES) == B and all((s - 8) % RPC == 0 for s in SIZES)
assert max(SIZES) == B_FULL + 8

# Each core reads (SIZES-8)//16 full chunks plus its own 8-row mini.
# Full chunk c covers rows [16c, 16c+16); cores whose full-chunk row
# count (SIZES-8) is <= 16c skip it.
_SKIP = {c: tuple(p for p in range(NCORES) if SIZES[p] - 8 <= RPC * c)
         for c in range(NCH)}
COND = {c: s for c, s in _SKIP.items() if s}          # chunk -> skip pids
TIERS = sorted(set(COND.values()))                    # distinct skip sets
NT = len(TIERS)

# Emission order: first 8 and the last slots unconditional, each
# conditional chunk followed by an unconditional one, leftovers last.
_UNC = [c for c in range(NCH) if c not in COND]
_CND = sorted(COND)
ORDER = list(_UNC[:8])
_rest = _UNC[8:]
for i, c in enumerate(_CND):
    ORDER.append(c)
    ORDER.extend(_rest[i: i + 1])
ORDER.extend(_rest[len(_CND):])
assert sorted(ORDER) == list(range(NCH))
assert all(c not in COND for c in ORDER[:8]) and ORDER[-1] not in COND

_BUILT = None
# test.py can inject {"trace": True, ...} here; harness path leaves it empty.
RUN_KWARGS = {}
LAST_RESULTS = None


def _build():
    from concourse import bacc, tile, mybir

    f32 = mybir.dt.float32
    f32r = mybir.dt.float32r
    nc = bacc.Bacc(
        "TRN2",
        target_bir_lowering=False,
        debug=False,
        enable_asserts=False,
        num_devices=NCORES,
    )
    t_in = nc.dram_tensor("t_shard", [NCH, 128, CW], f32r,
                          kind="ExternalInput").ap()
    # Column block i of 128 = the four q2-stationaries for tier i-1
    # (block 0 = unconditional): W_q2[p, m] = (m == 4*(p%8) + q2),
    # zeroed by the HOST on cores that skip the tier.  Per-core input
    # values do the masking.
    w_in = nc.dram_tensor("w_tiers", [128, 128 * (1 + NT) + 64], f32r,
                          kind="ExternalInput").ap()
    t_mini = nc.dram_tensor("t_mini", [128, 4096], f32r,
                            kind="ExternalInput").ap()
    out_t = nc.dram_tensor("out_t_part", [128, 512], f32,
                           kind="ExternalOutput").ap()

    with tile.TileContext(nc) as tc:
        with (
            tc.tile_pool(name="wconst", bufs=1) as wpool,
            tc.tile_pool(name="loads", bufs=4) as loads,
            tc.tile_pool(name="res", bufs=1) as res,
            tc.tile_pool(name="acc", bufs=1, space="PSUM") as ppool,
        ):
            wt = wpool.tile([128, 128 * (1 + NT) + 64], f32r)
            psum_t = ppool.tile([32, 2048], f32)
            stage = res.tile([128, 512], f32)
            tm = res.tile([128, 4096], f32r, name="tm")
            skip_cond = {}

            def wsel(c, q2):
                i = (1 + TIERS.index(COND[c])) if c in COND else 0
                o = 128 * i + 32 * q2
                return wt[:, o: o + 32]

            def fold(c, src, src_off, q2, bank, start, stop):
                # src cols [src_off, src_off+512) hold chunk cols
                # q2*2048 + bank*512 .. +512 -> psum[:, bank*512..],
                # rows 4*(p%8)+q2 (other rows += 0).
                nc.tensor.matmul(
                    psum_t[:, bank * 512: (bank + 1) * 512],
                    wsel(c, q2),
                    src[:, src_off: src_off + 512],
                    start=start,
                    stop=stop,
                )

            # The stationary load MUST precede the first chunk's
            # matmuls in program order (a later write would be a WAR
            # dep: the matmuls would run first, on uninitialized SBUF).
            # It rides the ACT ring so the SP ring's first chunk DMA
            # issue is not delayed.
            nc.scalar.dma_start(wt[:], w_in[:])

            def emit_setup():
                # Emitted after the first few t loads so the pid
                # register load (a ~950 ns sync-engine TENSOR_LOAD)
                # never delays the t stream's start; it is only needed
                # before the first conditional chunk at position 8.
                pid_sync = nc.sync.partition_id()
                for t in TIERS:
                    cs = None
                    for p in t:
                        es = pid_sync != p
                        cs = es if cs is None else cs * es
                    skip_cond[t] = cs

            for k, c in enumerate(ORDER):
                if k == 3:
                    emit_setup()
                if k == NCH - 1:
                    # 8-row MINI chunk first (2 MB, 16 KB-run layout:
                    # partition p holds mini row p//16, flat cols
                    # [(p%16)*4096, +4096)); psum row = 2*(p%16) +
                    # j//2048, so its two j-block stationaries are
                    # Wm_h[p, m] = (m == 2*(p%16) + h).  Every core
                    # reads its OWN last 8 rows here (per-core input
                    # data), so the DMA is unconditional - this is
                    # what lets core 6 carry 200 rows (12 full chunks
                    # + mini) with no padding waste.
                    nc.sync.dma_start(tm[:], t_mini[:])
                    wm_o = 128 * (1 + NT)
                    for h in range(2):
                        for b in range(4):
                            nc.tensor.matmul(
                                psum_t[:, b * 512: (b + 1) * 512],
                                wt[:, wm_o + 32 * h: wm_o + 32 * h + 32],
                                tm[:, h * 2048 + b * 512:
                                   h * 2048 + (b + 1) * 512],
                                start=False,
                                stop=False,
                            )
                    # Last chunk: three 1 MB quarter DMAs (q2 blocks
                    # 0-2) then q2=3 as TWO 512 KB eighths (banks 0-1,
                    # then banks 2-3).  A psum bank is final right
                    # after its q2=3 matmul, so the copies (DVE banks
                    # 0/2, ACT banks 1/3, running in parallel) chase
                    # the eighths, and the four [32, 512] output DMAs
                    # chase the copies on the now-idle SP ring.  Only
                    # the final eighth's 2 matmuls + copies + 128 KB of
                    # writes + receipt stay exposed past the stream's
                    # last byte.  (Engine program order matters: the
                    # out DMAs must NOT sit on the ACT queue ahead of
                    # the ACT copies.)
                    for q2 in range(3):
                        tq = res.tile([128, 2048], f32r, name=f"tq{q2}")
                        nc.sync.dma_start(
                            tq[:], t_in[c][:, q2 * 2048: (q2 + 1) * 2048])
                        for b in range(4):
                            fold(c, tq, b * 512, q2, b,
                                 start=False, stop=False)
                    for e8 in range(2):
                        te = res.tile([128, 1024], f32r, name=f"te{e8}")
                        o = 3 * 2048 + e8 * 1024
                        nc.sync.dma_start(te[:], t_in[c][:, o: o + 1024])
                        for b in (2 * e8, 2 * e8 + 1):
                            fold(c, te, (b - 2 * e8) * 512, 3, b,
                                 start=False, stop=True)
                            sl = slice(b * 512, (b + 1) * 512)
                            st = stage[32 * b: 32 * b + 32, :]
                            if b % 2 == 0:
                                nc.vector.tensor_copy(st, psum_t[:, sl])
                            else:
                                nc.scalar.copy(out=st, in_=psum_t[:, sl])
                    for b in range(4):
                        nc.sync.dma_start(out_t[32 * b: 32 * b + 32, :],
                                          stage[32 * b: 32 * b + 32, :])
                    continue
                tl = loads.tile([128, CW], f32r)
                if c in COND:
                    # Skipped on cores not owning these rows: the slot
                    # then holds stale (finite) data from an earlier
                    # chunk; the host-zeroed tier stationary
                    # neutralizes it (x * 0 == 0).
                    nc.sync.dma_start(tl[:], t_in[c], cond=skip_cond[COND[c]])
                else:
                    nc.sync.dma_start(tl[:], t_in[c])
                for q2 in range(4):
                    for b in range(4):
                        fold(c, tl, q2 * 2048 + b * 512, q2, b,
                             start=(k == 0 and q2 == 0), stop=False)

    nc.compile()
    return nc


def _get_built():
    global _BUILT
    if _BUILT is None:
        _BUILT = _build()
    return _BUILT


def kernel(h, r, t, w_i, w_j, w_k):
    global LAST_RESULTS
    from concourse import bass_utils

    nc = _get_built()
    t2 = np.ascontiguousarray(t, dtype=np.float32).reshape(B, FLAT)

    # Fold stationaries: W_q2[p, m] = (m == 4*(p%8) + q2), q2 = 0..3.
    w_q = np.zeros((128, 128), dtype=np.float32)  # horizontal concat of 4
    for q2 in range(4):
        w_q[np.arange(128), 32 * q2 + 4 * (np.arange(128) % 8) + q2] = 1.0

    # Mini-chunk stationaries: Wm_h[p, m] = (m == 2*(p%16) + h), h = 0, 1.
    w_m = np.zeros((128, 64), dtype=np.float32)
    for hh in range(2):
        w_m[np.arange(128), 32 * hh + 2 * (np.arange(128) % 16) + hh] = 1.0

    starts = np.concatenate([[0], np.cumsum(SIZES)])
    in_maps = []
    for c in range(NCORES):
        s, e = int(starts[c]), int(starts[c + 1])
        full = t2[s: e - 8]
        if full.shape[0] < B_FULL:
            full = np.concatenate(
                [full, np.zeros((B_FULL - full.shape[0], FLAT),
                                dtype=np.float32)])
        mini = t2[e - 8: e]
        w_tiers = np.concatenate(
            [w_q]
            + [w_q * (0.0 if c in t else 1.0) for t in TIERS]
            + [w_m],
            axis=1,
        )
        in_maps.append(
            {
                "t_shard": np.ascontiguousarray(full).reshape(NCH, 128, CW),
                "t_mini": np.ascontiguousarray(mini).reshape(128, 4096),
                "w_tiers": np.ascontiguousarray(w_tiers),
            }
        )
    results = bass_utils.run_bass_kernel_spmd(
        nc, in_maps, core_ids=list(range(NCORES)), **RUN_KWARGS
    )
    LAST_RESULTS = results

    # Staging tile rows 32b+m hold psum[m, 512b + u], and psum[m, u] =
    # sum_b t_loc[b, 2048m + u]: psum IS the flat batch sum.  Finish
    # the sum over cores on the host (f64), then unflatten to [N, D].
    # h/r sums are pure host work (1 MB each vs t's 512 MB).
    acc = np.zeros((128, 512), dtype=np.float64)
    for c in range(NCORES):
        acc += results.results[c]["out_t_part"]
    psum = acc.reshape(4, 32, 512).transpose(1, 0, 2).reshape(32, 2048)
    t_full = psum.reshape(FLAT).reshape(N, D)
    sum_h = h.astype(np.float64).sum(axis=0)
    sum_r = r.astype(np.float64).sum(axis=0)

    out = np.empty((N, 3 * D), dtype=np.float32)
    out[:, 0:D] = sum_h.astype(np.float32)[None, :]
    out[:, D: 2 * D] = sum_r.astype(np.float32)[None, :]
    out[:, 2 * D:] = t_full.astype(np.float32)
    return out
